# revision 27
# baseline (speedup 1.0000x reference)
# DeepSeek-style MoE PINN kernel for Trainium2 (Bass/Tile), 8-core data parallel.
#
# Math (per reference):
#   h = tanh([x,t] @ W_in + b_in)                        [N,64]
#   3x MoE layer:
#     sh = tanh(h @ sW1[e] + sb1[e]) for 2 shared experts
#     shared_out = sum_e sh[e] @ sW2[e] + sb2.sum(0)
#     logits = h @ routW + routb ; rw = softmax(logits)
#     top-2 mask (not renormalized): w = rw * mask
#     rh = tanh(h @ rW1[e] + rb1[e]) for 4 routed experts
#     r_out[e] = rh[e] @ rW2[e] + rb2[e]
#     routed_out = sum_e w[:,e] * r_out[e]
#     h = tanh(h + shared_out + routed_out)
#   y = h @ W_out + b_out
#
# Layout: feature-major h [64, tokens]; experts packed 2-wide on the matmul
# M dim (K=64), expert pairs K-packed (K=128) on the second matmul with
# router weights pre-multiplied into rh. Router runs token-major ([128 tok,
# 4 experts] PSUM via h-chunk-stationary matmuls); top-2 mask from pairwise
# min/max on DVE; masked weights transposed back via PE transpose and
# broadcast to [128, 512] scale tiles by selector matmuls. Biases enter as
# per-partition ACT bias columns (sb1/rb1/b_in/sb2) and an exp(routb)
# correction factor; all-zero rb2 skips its matmul.

import numpy as np

N_TOTAL = 262144
D = 64
L = 3
NCORES = 8
NPC = N_TOTAL // NCORES  # tokens per core (32768)
MB = 8192                # tokens per macro-batch (h ping/pong resident)
NMB = NPC // MB          # 4
TT = 512                 # tokens per tile
TPM = MB // TT           # 16 tiles per macro-batch
GRP = 4                  # tiles per router group
NG = TPM // GRP          # router groups per macro-batch

_CACHE = {}


def _build_module(with_rb2: bool, npc: int = NPC, mbsz: int = MB, ncores: int = NCORES):
    NPC = npc
    MB = mbsz
    NMB = NPC // MB
    TPM = MB // TT
    NG = TPM // GRP
    from contextlib import ExitStack

    import concourse.bass as bass
    import concourse.tile as tile
    from concourse import bacc, mybir

    f32 = mybir.dt.float32
    f16 = mybir.dt.float16
    AF = mybir.ActivationFunctionType
    OP = mybir.AluOpType

    nc = bacc.Bacc("TRN2", num_devices=ncores, debug=False, enable_asserts=False)

    xin = nc.dram_tensor("xin", [2, NPC], f16, kind="ExternalInput").ap()
    w1c = nc.dram_tensor("w1c", [L, 64, 384], f16, kind="ExternalInput").ap()
    w2c = nc.dram_tensor("w2c", [L, 128, 192], f16, kind="ExternalInput").ap()
    rtc = nc.dram_tensor("rtc", [L, 64, 4], f16, kind="ExternalInput").ap()
    rb2c = nc.dram_tensor("rb2c", [L, 2, 128, 64], f16, kind="ExternalInput").ap()
    # bias columns: 0: b_in; 1+3l: sb1[l]; 2+3l: rb1[l,0:2]; 3+3l: rb1[l,2:4];
    # 10+l: sb2[l].sum(0); 13: b_out (replicated)
    bcl = nc.dram_tensor("bcl", [128, 14], f32, kind="ExternalInput").ap()
    # exp(routb[l]) replicated over partitions
    crw = nc.dram_tensor("crw", [128, 4 * L], f32, kind="ExternalInput").ap()
    winc = nc.dram_tensor("winc", [2, 64], f16, kind="ExternalInput").ap()
    woutc = nc.dram_tensor("woutc", [64, 32], f16, kind="ExternalInput").ap()
    idc = nc.dram_tensor("idc", [128, 128], f32, kind="ExternalInput").ap()
    y = nc.dram_tensor("y", [NPC, 1], f32, kind="ExternalOutput").ap()
    yv = y.rearrange("(a b) o -> a (b o)", b=min(4 * TT, NPC))  # head output rows

    with tile.TileContext(nc) as tc, ExitStack() as ctx:
        singles = ctx.enter_context(tc.tile_pool(name="singles", bufs=1))
        hpool = ctx.enter_context(tc.tile_pool(name="hpool", bufs=2))
        xtpool = ctx.enter_context(tc.tile_pool(name="xtpool", bufs=2))
        stage = ctx.enter_context(tc.tile_pool(name="stage", bufs=3))
        rstage = ctx.enter_context(tc.tile_pool(name="rstage", bufs=2))
        small = ctx.enter_context(tc.tile_pool(name="small", bufs=4))
        ps = ctx.enter_context(tc.tile_pool(name="ps", bufs=1, space="PSUM"))
        dpool = ctx.enter_context(tc.tile_pool(name="dpool", bufs=4, space="DRAM"))

        # --- constants to SBUF (once) ---
        w1_sb = []
        w2_sb = []
        rt_sb = []
        rb2_sb = []
        for l in range(L):
            wl = singles.tile([64, 384], f16, name=f"w1l{l}")
            nc.sync.dma_start(out=wl, in_=w1c[l])
            w1_sb.append(wl)
            w2l = singles.tile([128, 192], f16, name=f"w2l{l}")
            nc.sync.dma_start(out=w2l, in_=w2c[l])
            w2_sb.append(w2l)
            rtl = singles.tile([64, 4], f16, name=f"rtl{l}")
            nc.sync.dma_start(out=rtl, in_=rtc[l])
            rt_sb.append(rtl)
            if with_rb2:
                rbl = singles.tile([128, 2, 64], f16, name=f"rbl{l}")
                nc.sync.dma_start(
                    out=rbl, in_=rb2c[l].rearrange("a p f -> p a f")
                )
                rb2_sb.append(rbl)
        bcl_sb = singles.tile([128, 14], f32)
        nc.sync.dma_start(out=bcl_sb, in_=bcl)
        crw_sb = singles.tile([128, 4 * L], f32)
        nc.sync.dma_start(out=crw_sb, in_=crw)
        win_sb = singles.tile([2, 64], f16)
        nc.sync.dma_start(out=win_sb, in_=winc)
        wout_sb = singles.tile([64, 32], f16)
        nc.sync.dma_start(out=wout_sb, in_=woutc)
        id_sb = singles.tile([128, 128], f32)
        nc.sync.dma_start(out=id_sb, in_=idc)

        for mb in range(NMB):
            mbs = slice(mb * MB, (mb + 1) * MB)
            xt = xtpool.tile([2, MB], f16, tag="xt")
            nc.sync.dma_start(out=xt, in_=xin[:, mbs])

            # ---- layer 0: h = tanh(W_in^T @ [x;t] + b_in) ----
            h = hpool.tile([64, MB], f16, tag="h")
            for t in range(TPM):
                tsl = bass.ts(t, TT)
                p0 = ps.tile([64, TT], f32, tag="po", bufs=1, padded_shape=[128, TT])
                nc.tensor.matmul(p0, win_sb, xt[:, tsl], start=True, stop=True)
                nc.scalar.activation(
                    h[:, tsl], p0, AF.Tanh, bias=bcl_sb[0:64, 0:1]
                )

            # ---- MoE layers ----
            for l in range(L):
                hn = hpool.tile([64, MB], f16, tag="h")
                for g in range(NG):
                    # ===== router for tiles [4g, 4g+4): token-major =====
                    plg = ps.tile([128, GRP * 16], f32, tag="lgwt", bufs=2,
                                  padded_shape=[128, TT])
                    for tg in range(GRP):
                        t = g * GRP + tg
                        for c in range(4):
                            hc = h[:, t * TT + c * 128 : t * TT + (c + 1) * 128]
                            nc.tensor.matmul(
                                plg[:, tg * 16 + c * 4 : tg * 16 + (c + 1) * 4],
                                hc,
                                rt_sb[l][:, :],
                                start=True,
                                stop=True,
                            )
                    ee = rstage.tile([128, GRP * 16], f32, tag="ee")
                    nc.scalar.activation(ee, plg, AF.Exp)
                    # multiply by exp(routb) and sum over experts
                    e3 = ee.rearrange("p (q e) -> p q e", e=4)
                    crw_b = (
                        crw_sb[:, 4 * l : 4 * l + 4]
                        .unsqueeze(1)
                        .broadcast_to((128, GRP * 4, 4))
                    )
                    ec = rstage.tile([128, GRP * 16], f32, tag="ec")
                    ec3 = ec.rearrange("p (q e) -> p q e", e=4)
                    nc.vector.tensor_mul(ec3, e3, crw_b)
                    ss = small.tile([128, GRP * 4], f32, tag="ss")
                    nc.vector.reduce_sum(ss, ec3, axis=mybir.AxisListType.X)
                    rs = small.tile([128, GRP * 4], f32, tag="rs")
                    nc.vector.reciprocal(rs, ss)
                    rw = rstage.tile([128, GRP * 16], f32, tag="rw")
                    rs_b = rs.unsqueeze(2).broadcast_to((128, GRP * 4, 4))
                    r3 = rw.rearrange("p (q e) -> p q e", e=4)
                    nc.vector.tensor_mul(r3, ec3, rs_b)
                    m1 = small.tile([128, GRP * 4], f32, tag="m1")
                    nc.vector.tensor_tensor(m1, r3[:, :, 0], r3[:, :, 1], op=OP.max)
                    n1 = small.tile([128, GRP * 4], f32, tag="n1")
                    nc.vector.tensor_tensor(n1, r3[:, :, 0], r3[:, :, 1], op=OP.min)
                    m2 = small.tile([128, GRP * 4], f32, tag="m2")
                    nc.vector.tensor_tensor(m2, r3[:, :, 2], r3[:, :, 3], op=OP.max)
                    n2 = small.tile([128, GRP * 4], f32, tag="n2")
                    nc.vector.tensor_tensor(n2, r3[:, :, 2], r3[:, :, 3], op=OP.min)
                    t1 = small.tile([128, GRP * 4], f32, tag="t1")
                    nc.vector.tensor_tensor(t1, m1, m2, op=OP.min)
                    t2 = small.tile([128, GRP * 4], f32, tag="t2")
                    nc.vector.tensor_tensor(t2, n1, n2, op=OP.max)
                    snd = small.tile([128, GRP * 4], f32, tag="snd")
                    nc.vector.tensor_tensor(snd, t1, t2, op=OP.max)
                    mk = rstage.tile([128, GRP * 16], f32, tag="mk")
                    snd_b = snd.unsqueeze(2).broadcast_to((128, GRP * 4, 4))
                    nc.vector.tensor_tensor(
                        mk.rearrange("p (q e) -> p q e", e=4), r3, snd_b, op=OP.is_ge
                    )
                    wf = rstage.tile([128, GRP * 16], f32, tag="wf")
                    nc.vector.tensor_mul(wf, rw, mk)

                    # ===== main per-tile compute =====
                    for tg in range(GRP):
                        t = g * GRP + tg
                        tsl = bass.ts(t, TT)
                        hs = h[:, tsl]
                        # W1 stage: 2 shared + 4 routed first-layer matmuls
                        psh = ps.tile([128, TT], f32, tag="w1", bufs=5)
                        nc.tensor.matmul(
                            psh, w1_sb[l][:, 0:128], hs, start=True, stop=True
                        )
                        pr1 = ps.tile([128, TT], f32, tag="w1", bufs=5)
                        nc.tensor.matmul(
                            pr1, w1_sb[l][:, 128:256], hs, start=True, stop=True
                        )
                        pr2 = ps.tile([128, TT], f32, tag="w1", bufs=5)
                        nc.tensor.matmul(
                            pr2, w1_sb[l][:, 256:384], hs, start=True, stop=True
                        )
                        sh = stage.tile([128, 3 * TT], f16, tag="sh")
                        nc.scalar.activation(
                            sh[:, 0:TT], psh, AF.Tanh,
                            bias=bcl_sb[:, 1 + 3 * l : 2 + 3 * l],
                        )
                        nc.scalar.activation(
                            sh[:, TT : 2 * TT], pr1, AF.Tanh,
                            bias=bcl_sb[:, 2 + 3 * l : 3 + 3 * l],
                        )
                        nc.scalar.activation(
                            sh[:, 2 * TT : 3 * TT], pr2, AF.Tanh,
                            bias=bcl_sb[:, 3 + 3 * l : 4 + 3 * l],
                        )

                        # transpose masked weights [128,16] -> [16,128]:
                        # row (4c+e) = chunk c's 128 tokens for expert e
                        pwt = ps.tile([16, 128], f32, tag="lgwt", bufs=2,
                                      padded_shape=[128, TT])
                        nc.tensor.transpose(
                            pwt, wf[:, tg * 16 : (tg + 1) * 16], id_sb
                        )
                        wts = rstage.tile([16, 128], f16, tag="wts")
                        nc.vector.tensor_copy(wts, pwt)
                        # bounce w rows through DRAM, then broadcast-read with
                        # step-0 partition + chunk-strided APs (1 DMA / expert)
                        wdr = dpool.tile([16, 128], f16, tag="wdr")
                        nc.sync.dma_start(out=wdr, in_=wts)
                        pwsb = stage.tile([128, 2 * TT], f16, tag="pwsb")
                        def _brd(e):
                            ap = wdr[0:1, 0:1]
                            return bass.AP(
                                tensor=ap.tensor,
                                offset=ap.offset + e * 128,
                                ap=[[0, 64], [512, 4], [1, 128]],
                            )
                        nc.sync.dma_start(out=pwsb[0:64, 0:TT], in_=_brd(0))
                        nc.sync.dma_start(out=pwsb[64:128, 0:TT], in_=_brd(1))
                        nc.sync.dma_start(out=pwsb[0:64, TT : 2 * TT], in_=_brd(2))
                        nc.sync.dma_start(out=pwsb[64:128, TT : 2 * TT], in_=_brd(3))
                        rsc = stage.tile([128, 2 * TT], f16, tag="rsc")
                        nc.vector.tensor_mul(rsc, sh[:, TT : 3 * TT], pwsb)

                        # W2 stage: accumulate shared + routed (+ rb2)
                        po = ps.tile([64, TT], f32, tag="po", bufs=1,
                                     padded_shape=[128, TT])
                        nc.tensor.matmul(
                            po, w2_sb[l][:, 0:64], sh[:, 0:TT],
                            start=True, stop=False,
                        )
                        nc.tensor.matmul(
                            po, w2_sb[l][:, 64:128], rsc[:, 0:TT],
                            start=False, stop=False,
                        )
                        nc.tensor.matmul(
                            po, w2_sb[l][:, 128:192], rsc[:, TT : 2 * TT],
                            start=False, stop=not with_rb2,
                        )
                        if with_rb2:
                            nc.tensor.matmul(
                                po, rb2_sb[l][:, 0, :], pwsb[:, 0:TT],
                                start=False, stop=False,
                            )
                            nc.tensor.matmul(
                                po, rb2_sb[l][:, 1, :], pwsb[:, TT : 2 * TT],
                                start=False, stop=True,
                            )
                        # residual add on DVE, then tanh
                        ha = stage.tile([64, TT], f32, tag="ha")
                        nc.vector.tensor_add(ha, po, h[:, tsl])
                        nc.scalar.activation(
                            hn[:, tsl], ha, AF.Tanh,
                            bias=bcl_sb[0:64, 10 + l : 11 + l],
                        )
                h = hn

            # ---- head: pack 4 tiles' [1,TT] outputs at partitions {0,32,64,96}
            for hg in range(TPM // 4):
                py = ps.tile([128, TT], f32, tag="po", bufs=1)
                for j in range(4):
                    t = hg * 4 + j
                    tsl = bass.ts(t, TT)
                    nc.tensor.matmul(
                        py[32 * j : 32 * j + 32, :], wout_sb, h[:, tsl],
                        start=True, stop=True, tile_position=(0, 32 * j),
                    )
                ysb = rstage.tile([128, TT], f32, tag="ysb")
                nc.vector.tensor_scalar_add(ysb, py, bcl_sb[:, 13:14])
                yrow = ysb.rearrange("(a b) f -> a b f", b=32)[:, 0, :]  # [4, TT]
                nc.sync.dma_start(
                    out=yv[mb * (TPM // 4) + hg : mb * (TPM // 4) + hg + 1, :],
                    in_=yrow,
                )

    nc.compile()
    return nc


HMB = MB // 2      # tokens per macro-batch half (4096)
TPH = HMB // TT    # ptiles per macro-batch (8); each ptile = 1024 tokens
PGRP = 4           # ptiles per router group
NPG = TPH // PGRP  # router groups per macro-batch (2)


def _build_fast(npc: int = NPC, ncores: int = NCORES, use_fp16: bool = True):
    """Fast path for the all-zero-bias case (the shipped reference).

    Dual-half layout: each [128, x] tile holds tokens of macro-batch half A
    on partitions 0-63 and half B on partitions 64-127 (features 0-63 each).
    W1 / router / input / head matmuls run as row-group-concurrent pairs
    (tile_position rows 0 and 64) with weights duplicated on both partition
    halves. The three W1 outputs per half land in one [128, 3072] PSUM
    6-bank tile -> a single fused tanh. Router weight broadcast goes
    PE-transpose-free: wf -> DMA-transpose -> DRAM bounce ->
    partition-step-0 broadcast reads.
    """
    NPC = npc
    NMB = NPC // MB
    from contextlib import ExitStack

    import concourse.bass as bass
    import concourse.tile as tile
    from concourse import bacc, mybir

    f32 = mybir.dt.float32
    bf16 = mybir.dt.float16 if use_fp16 else mybir.dt.bfloat16
    AF = mybir.ActivationFunctionType
    OP = mybir.AluOpType

    nc = bacc.Bacc("TRN2", num_devices=ncores, debug=False, enable_asserts=False)

    xin = nc.dram_tensor("xin", [4, NPC // 2], bf16, kind="ExternalInput").ap()
    w1c = nc.dram_tensor("w1c", [L, 128, 384], bf16, kind="ExternalInput").ap()
    w2c = nc.dram_tensor("w2c", [L, 128, 192], bf16, kind="ExternalInput").ap()
    rtc = nc.dram_tensor("rtc", [L, 128, 4], bf16, kind="ExternalInput").ap()
    winc = nc.dram_tensor("winc", [128, 64], bf16, kind="ExternalInput").ap()
    woutc = nc.dram_tensor("woutc", [128, 32], bf16, kind="ExternalInput").ap()
    y = nc.dram_tensor("y", [NPC, 1], f32, kind="ExternalOutput").ap()

    with tile.TileContext(nc) as tc, ExitStack() as ctx:
        singles = ctx.enter_context(tc.tile_pool(name="singles", bufs=1))
        hpool = ctx.enter_context(tc.tile_pool(name="hpool", bufs=2))
        xtpool = ctx.enter_context(tc.tile_pool(name="xtpool", bufs=2))
        stage = ctx.enter_context(tc.tile_pool(name="stage", bufs=3))
        rstage = ctx.enter_context(tc.tile_pool(name="rstage", bufs=2))
        small = ctx.enter_context(tc.tile_pool(name="small", bufs=4))
        ps = ctx.enter_context(tc.tile_pool(name="ps", bufs=1, space="PSUM"))
        dpool = ctx.enter_context(tc.tile_pool(name="dpool", bufs=4, space="DRAM"))

        w1_sb = []
        w2_sb = []
        rt_sb = []
        for l in range(L):
            wl = singles.tile([128, 384], bf16, name=f"w1l{l}")
            nc.sync.dma_start(out=wl, in_=w1c[l])
            w1_sb.append(wl)
            w2l = singles.tile([128, 192], bf16, name=f"w2l{l}")
            nc.sync.dma_start(out=w2l, in_=w2c[l])
            w2_sb.append(w2l)
            rtl = singles.tile([128, 4], bf16, name=f"rtl{l}")
            nc.sync.dma_start(out=rtl, in_=rtc[l])
            rt_sb.append(rtl)
        win_sb = singles.tile([128, 64], bf16)
        nc.sync.dma_start(out=win_sb, in_=winc)
        wout_sb = singles.tile([128, 32], bf16)
        nc.sync.dma_start(out=wout_sb, in_=woutc)

        for mb in range(NMB):
            mbs = slice(mb * HMB, (mb + 1) * HMB)
            xt = xtpool.tile([66, HMB], bf16, tag="xt")
            nc.sync.dma_start(out=xt[0:2, :], in_=xin[0:2, mbs])
            nc.sync.dma_start(out=xt[64:66, :], in_=xin[2:4, mbs])

            # ---- layer 0: h = tanh(W_in^T @ [x;t]), A|B halves ----
            hb = hpool.tile([128, HMB], bf16, tag="hb")
            for t in range(TPH):
                tsl = bass.ts(t, TT)
                p0 = ps.tile([128, TT], f32, tag="po", bufs=1)
                nc.tensor.matmul(p0[0:64, :], win_sb[0:2, :], xt[0:2, tsl],
                                 start=True, stop=True, tile_position=(0, 0))
                nc.tensor.matmul(p0[64:128, :], win_sb[64:66, :], xt[64:66, tsl],
                                 start=True, stop=True, tile_position=(64, 64))
                nc.scalar.activation(hb[:, tsl], p0, AF.Tanh)

            # ---- MoE layers ----
            for l in range(L):
                hbn = hpool.tile([128, HMB], bf16, tag="hb")
                for g in range(NPG):
                    # ===== router for ptile group: token-major logits =====
                    plg = ps.tile([128, PGRP * 32], f32, tag="lgwt", bufs=1,
                                  padded_shape=[128, TT])
                    for tg in range(PGRP):
                        t = g * PGRP + tg
                        for c in range(4):
                            csl = slice(t * TT + c * 128, t * TT + (c + 1) * 128)
                            nc.tensor.matmul(
                                plg[:, tg * 32 + c * 4 : tg * 32 + (c + 1) * 4],
                                hb[0:64, csl], rt_sb[l][0:64, :],
                                start=True, stop=True, tile_position=(0, 0),
                            )
                            nc.tensor.matmul(
                                plg[:, tg * 32 + 16 + c * 4 : tg * 32 + 16 + (c + 1) * 4],
                                hb[64:128, csl], rt_sb[l][64:128, :],
                                start=True, stop=True, tile_position=(64, 0),
                            )
                    ee = rstage.tile([128, PGRP * 32], f32, tag="ee")
                    nc.scalar.activation(ee, plg, AF.Exp)
                    e4 = ee.rearrange("p (q e) -> p q e", e=4)
                    ss = small.tile([128, PGRP * 8], f32, tag="ss")
                    nc.vector.reduce_sum(ss, e4, axis=mybir.AxisListType.X)
                    rs = small.tile([128, PGRP * 8], f32, tag="rs")
                    nc.vector.reciprocal(rs, ss)
                    rw = rstage.tile([128, PGRP * 32], f32, tag="rw")
                    rs_b = rs.unsqueeze(2).broadcast_to((128, PGRP * 8, 4))
                    r4 = rw.rearrange("p (q e) -> p q e", e=4)
                    nc.vector.tensor_mul(r4, e4, rs_b)
                    m1 = small.tile([128, PGRP * 8], f32, tag="m1")
                    nc.vector.tensor_tensor(m1, r4[:, :, 0], r4[:, :, 1], op=OP.max)
                    n1 = small.tile([128, PGRP * 8], f32, tag="n1")
                    nc.vector.tensor_tensor(n1, r4[:, :, 0], r4[:, :, 1], op=OP.min)
                    m2 = small.tile([128, PGRP * 8], f32, tag="m2")
                    nc.vector.tensor_tensor(m2, r4[:, :, 2], r4[:, :, 3], op=OP.max)
                    n2 = small.tile([128, PGRP * 8], f32, tag="n2")
                    nc.vector.tensor_tensor(n2, r4[:, :, 2], r4[:, :, 3], op=OP.min)
                    t1 = small.tile([128, PGRP * 8], f32, tag="t1")
                    nc.vector.tensor_tensor(t1, m1, m2, op=OP.min)
                    t2 = small.tile([128, PGRP * 8], f32, tag="t2")
                    nc.vector.tensor_tensor(t2, n1, n2, op=OP.max)
                    snd = small.tile([128, PGRP * 8], f32, tag="snd")
                    nc.vector.tensor_tensor(snd, t1, t2, op=OP.max)
                    mk = rstage.tile([128, PGRP * 32], f32, tag="mk")
                    snd_b = snd.unsqueeze(2).broadcast_to((128, PGRP * 8, 4))
                    nc.vector.tensor_tensor(
                        mk.rearrange("p (q e) -> p q e", e=4), r4, snd_b, op=OP.is_ge
                    )
                    wf = rstage.tile([128, PGRP * 32], bf16, tag="wf")
                    nc.vector.tensor_mul(wf, rw, mk)
                    # token-major -> row-per-(tile,half,chunk,expert) via DMA
                    # transpose, then DRAM bounce for broadcast reads
                    wtg = rstage.tile([128, 128], bf16, tag="wtg")
                    nc.sync.dma_start_transpose(wtg, wf)
                    wdr = dpool.tile([128, 128], bf16, tag="wdr")
                    nc.sync.dma_start(out=wdr, in_=wtg)

                    # ===== per-ptile compute =====
                    for tg in range(PGRP):
                        t = g * PGRP + tg
                        tsl = bass.ts(t, TT)
                        # broadcast masked weights to [128, 4*TT] scale tiles
                        pwsb = stage.tile([128, 4 * TT], bf16, tag="pwsb")
                        def _brd(half, e):
                            ap = wdr[0:1, 0:1]
                            return bass.AP(
                                tensor=ap.tensor,
                                offset=ap.offset + (tg * 32 + half * 16 + e) * 128,
                                ap=[[0, 64], [512, 4], [1, 128]],
                            )
                        for half in range(2):
                            for pr in range(2):
                                blk = half * 2 + pr
                                nc.sync.dma_start(
                                    out=pwsb[0:64, blk * TT : (blk + 1) * TT],
                                    in_=_brd(half, 2 * pr),
                                )
                                nc.sync.dma_start(
                                    out=pwsb[64:128, blk * TT : (blk + 1) * TT],
                                    in_=_brd(half, 2 * pr + 1),
                                )

                        # W1 stage: 3 row-group-concurrent pairs -> 6 banks
                        psw = ps.tile([128, 3072], f32, tag="w1", bufs=1)
                        for j in range(3):
                            jsl = slice(j * 128, (j + 1) * 128)
                            nc.tensor.matmul(
                                psw[:, j * TT : (j + 1) * TT],
                                w1_sb[l][0:64, jsl], hb[0:64, tsl],
                                start=True, stop=True, tile_position=(0, 0),
                            )
                            nc.tensor.matmul(
                                psw[:, 1536 + j * TT : 1536 + (j + 1) * TT],
                                w1_sb[l][64:128, jsl], hb[64:128, tsl],
                                start=True, stop=True, tile_position=(64, 0),
                            )
                        sh = stage.tile([128, 3072], bf16, tag="sh")
                        nc.scalar.activation(sh, psw, AF.Tanh)

                        rsc = stage.tile([128, 2048], bf16, tag="rsc")
                        nc.vector.tensor_mul(
                            rsc[:, 0:1024], sh[:, TT : 3 * TT], pwsb[:, 0 : 2 * TT]
                        )
                        nc.vector.tensor_mul(
                            rsc[:, 1024:2048], sh[:, 2048:3072],
                            pwsb[:, 2 * TT : 4 * TT],
                        )

                        # W2 stage: A -> po[0:64], B -> po[64:128]
                        po = ps.tile([128, TT], f32, tag="po", bufs=1)
                        nc.tensor.matmul(
                            po[0:64, :], w2_sb[l][:, 0:64], sh[:, 0:TT],
                            start=True, stop=False, tile_position=(0, 0),
                        )
                        nc.tensor.matmul(
                            po[0:64, :], w2_sb[l][:, 64:128], rsc[:, 0:TT],
                            start=False, stop=False, tile_position=(0, 0),
                        )
                        nc.tensor.matmul(
                            po[0:64, :], w2_sb[l][:, 128:192], rsc[:, TT : 2 * TT],
                            start=False, stop=True, tile_position=(0, 0),
                        )
                        nc.tensor.matmul(
                            po[64:128, :], w2_sb[l][:, 0:64], sh[:, 1536:2048],
                            start=True, stop=False, tile_position=(0, 64),
                        )
                        nc.tensor.matmul(
                            po[64:128, :], w2_sb[l][:, 64:128], rsc[:, 2 * TT : 3 * TT],
                            start=False, stop=False, tile_position=(0, 64),
                        )
                        nc.tensor.matmul(
                            po[64:128, :], w2_sb[l][:, 128:192], rsc[:, 3 * TT : 4 * TT],
                            start=False, stop=True, tile_position=(0, 64),
                        )
                        # residual add on DVE, tanh back to 16-bit h
                        ha = stage.tile([128, TT], f32, tag="ha")
                        nc.vector.tensor_add(ha, po, hb[:, tsl])
                        nc.scalar.activation(hbn[:, tsl], ha, AF.Tanh)
                hb = hbn

            # ---- head: 2 ptiles per [128, TT] PSUM tile, direct DMA out ----
            for hg in range(TPH // 2):
                py = ps.tile([128, TT], f32, tag="po", bufs=1)
                for j in range(2):
                    t = hg * 2 + j
                    tsl = bass.ts(t, TT)
                    nc.tensor.matmul(
                        py[64 * j : 64 * j + 32, :], wout_sb[0:64, :], hb[0:64, tsl],
                        start=True, stop=True, tile_position=(0, 64 * j),
                    )
                    nc.tensor.matmul(
                        py[64 * j + 32 : 64 * j + 64, :], wout_sb[64:128, :],
                        hb[64:128, tsl],
                        start=True, stop=True, tile_position=(64, 64 * j + 32),
                    )
                ysb = rstage.tile([128, TT], f32, tag="ysb")
                nc.vector.tensor_copy(ysb, py)
                yrow = ysb.rearrange("(a b) f -> a b f", b=32)[:, 0, :]  # [4, TT]
                yt = y[0:1, 0:1]
                ydst = bass.AP(
                    tensor=yt.tensor,
                    offset=yt.offset + mb * MB + hg * 2 * TT,
                    ap=[[TT, 2], [HMB, 2], [1, TT]],
                )
                nc.sync.dma_start(out=ydst, in_=yrow)

    nc.compile()
    return nc


def _prep_host(inputs):
    f = np.float32
    bf = np.float16
    x = np.asarray(inputs["x"], f).reshape(-1)
    t = np.asarray(inputs["t"], f).reshape(-1)
    W_in = np.asarray(inputs["W_in"], f)
    b_in = np.asarray(inputs["b_in"], f)
    sW1 = np.asarray(inputs["sW1"], f)
    sb1 = np.asarray(inputs["sb1"], f)
    sW2 = np.asarray(inputs["sW2"], f)
    sb2 = np.asarray(inputs["sb2"], f)
    rW1 = np.asarray(inputs["rW1"], f)
    rb1 = np.asarray(inputs["rb1"], f)
    rW2 = np.asarray(inputs["rW2"], f)
    rb2 = np.asarray(inputs["rb2"], f)
    routW = np.asarray(inputs["routW"], f)
    routb = np.asarray(inputs["routb"], f)
    W_out = np.asarray(inputs["W_out"], f)
    b_out = np.asarray(inputs["b_out"], f)

    w1c = np.zeros((L, 64, 384), f)
    w2c = np.zeros((L, 128, 192), f)
    rtc = np.zeros((L, 64, 4), f)
    for l in range(L):
        w1c[l, :, 0:128] = np.transpose(sW1[l], (1, 0, 2)).reshape(64, 128)
        w1c[l, :, 128:256] = np.transpose(rW1[l, 0:2], (1, 0, 2)).reshape(64, 128)
        w1c[l, :, 256:384] = np.transpose(rW1[l, 2:4], (1, 0, 2)).reshape(64, 128)
        w2c[l, :, 0:64] = sW2[l].reshape(128, 64)
        w2c[l, :, 64:128] = rW2[l, 0:2].reshape(128, 64)
        w2c[l, :, 128:192] = rW2[l, 2:4].reshape(128, 64)
        rtc[l] = routW[l]
    rb2c = np.zeros((L, 2, 128, 64), f)
    for l in range(L):
        for half in range(2):
            for eb in range(2):
                e = half * 2 + eb
                rb2c[l, half, eb * 64 : (eb + 1) * 64, :] = rb2[l, e][None, :] / 64.0
    bcl = np.zeros((128, 14), f)
    bcl[0:64, 0] = b_in
    for l in range(L):
        bcl[:, 1 + 3 * l] = sb1[l].reshape(128)
        bcl[:, 2 + 3 * l] = rb1[l, 0:2].reshape(128)
        bcl[:, 3 + 3 * l] = rb1[l, 2:4].reshape(128)
        bcl[0:64, 10 + l] = sb2[l].sum(0)
    bcl[:, 13] = b_out[0]
    crw = np.zeros((128, 4 * L), f)
    for l in range(L):
        crw[:, 4 * l : 4 * l + 4] = np.exp(routb[l])[None, :]
    winc = np.ascontiguousarray(W_in)  # [2, 64]
    woutc = np.ascontiguousarray(np.repeat(W_out, 32, axis=1))  # [64, 32]
    idc = np.eye(128, dtype=f)

    shared = {
        "w1c": w1c.astype(bf), "w2c": w2c.astype(bf), "rtc": rtc.astype(bf),
        "rb2c": rb2c.astype(bf), "bcl": bcl, "crw": crw,
        "winc": winc.astype(bf), "woutc": woutc.astype(bf), "idc": idc,
    }
    in_maps = []
    for c in range(NCORES):
        sl = slice(c * NPC, (c + 1) * NPC)
        xin = np.stack([x[sl], t[sl]], 0)  # [2, NPC]
        in_maps.append({"xin": np.ascontiguousarray(xin).astype(bf), **shared})
    return in_maps, bool(np.any(rb2 != 0.0))


def _prep_fast(inputs, use_fp16=True):
    f = np.float32
    bf = np.float16 if use_fp16 else None
    if bf is None:
        import ml_dtypes

        bf = ml_dtypes.bfloat16
    x = np.asarray(inputs["x"], f).reshape(-1)
    t = np.asarray(inputs["t"], f).reshape(-1)
    W_in = np.asarray(inputs["W_in"], f)
    sW1 = np.asarray(inputs["sW1"], f)
    sW2 = np.asarray(inputs["sW2"], f)
    rW1 = np.asarray(inputs["rW1"], f)
    rW2 = np.asarray(inputs["rW2"], f)
    routW = np.asarray(inputs["routW"], f)
    W_out = np.asarray(inputs["W_out"], f)

    w1c = np.zeros((L, 128, 384), f)
    w2c = np.zeros((L, 128, 192), f)
    rtc = np.zeros((L, 128, 4), f)
    for l in range(L):
        w1c[l, 0:64, 0:128] = np.transpose(sW1[l], (1, 0, 2)).reshape(64, 128)
        w1c[l, 0:64, 128:256] = np.transpose(rW1[l, 0:2], (1, 0, 2)).reshape(64, 128)
        w1c[l, 0:64, 256:384] = np.transpose(rW1[l, 2:4], (1, 0, 2)).reshape(64, 128)
        w1c[l, 64:128] = w1c[l, 0:64]
        w2c[l, :, 0:64] = sW2[l].reshape(128, 64)
        w2c[l, :, 64:128] = rW2[l, 0:2].reshape(128, 64)
        w2c[l, :, 128:192] = rW2[l, 2:4].reshape(128, 64)
        rtc[l, 0:64] = routW[l]
        rtc[l, 64:128] = routW[l]
    winc = np.zeros((128, 64), f)
    winc[0:2] = W_in
    winc[64:66] = W_in
    woutc = np.zeros((128, 32), f)
    woutc[0:64] = np.repeat(W_out, 32, axis=1)
    woutc[64:128] = woutc[0:64]

    shared = {
        "w1c": w1c.astype(bf), "w2c": w2c.astype(bf), "rtc": rtc.astype(bf),
        "winc": winc.astype(bf), "woutc": woutc.astype(bf),
    }
    in_maps = []
    for c in range(NCORES):
        sl = slice(c * NPC, (c + 1) * NPC)
        # split each macro-batch into halves A/B: rows 0-1 = (x,t) of A
        # tokens, rows 2-3 = B tokens
        xv = x[sl].reshape(NPC // MB, 2, HMB)
        tv = t[sl].reshape(NPC // MB, 2, HMB)
        xin = np.stack([
            xv[:, 0, :].reshape(-1), tv[:, 0, :].reshape(-1),
            xv[:, 1, :].reshape(-1), tv[:, 1, :].reshape(-1),
        ], 0)  # [4, NPC//2]
        in_maps.append({"xin": np.ascontiguousarray(xin).astype(bf), **shared})
    return in_maps


def _fast_ok(inputs):
    for k in ("b_in", "sb1", "sb2", "rb1", "rb2", "routb", "b_out"):
        if np.any(np.asarray(inputs[k], np.float32) != 0.0):
            return False
    return True


def _get_module(kind, *args):
    key = (kind,) + args
    if key not in _CACHE:
        if kind == "fast":
            _CACHE[key] = _build_fast(use_fp16=args[0])
        else:
            _CACHE[key] = _build_module(*args)
    return _CACHE[key]


def _run(inputs, trace=False, use_fp16=True):
    from concourse.bass_utils import run_bass_kernel_spmd

    if _fast_ok(inputs):
        in_maps = _prep_fast(inputs, use_fp16=use_fp16)
        nc = _get_module("fast", use_fp16)
    else:
        in_maps, with_rb2 = _prep_host(inputs)
        nc = _get_module("gen", with_rb2)
    res = run_bass_kernel_spmd(
        nc, in_maps, core_ids=list(range(NCORES)), trace=trace
    )
    yy = np.concatenate([r["y"] for r in res.results], 0).astype(np.float32)
    return yy, res


def kernel(**inputs) -> np.ndarray:
    yy, _ = _run(inputs, trace=False)
    return yy



# revision 34
# speedup vs baseline: 1.3320x; 1.3320x over previous
# DeepSeek-style MoE PINN kernel for Trainium2 (Bass/Tile), 8-core data parallel.
#
# Math (per reference):
#   h = tanh([x,t] @ W_in + b_in)                        [N,64]
#   3x MoE layer:
#     sh = tanh(h @ sW1[e] + sb1[e]) for 2 shared experts
#     shared_out = sum_e sh[e] @ sW2[e] + sb2.sum(0)
#     logits = h @ routW + routb ; rw = softmax(logits)
#     top-2 mask (not renormalized): w = rw * mask
#     rh = tanh(h @ rW1[e] + rb1[e]) for 4 routed experts
#     r_out[e] = rh[e] @ rW2[e] + rb2[e]
#     routed_out = sum_e w[:,e] * r_out[e]
#     h = tanh(h + shared_out + routed_out)
#   y = h @ W_out + b_out
#
# Layout: feature-major h [64, tokens]; experts packed 2-wide on the matmul
# M dim (K=64), expert pairs K-packed (K=128) on the second matmul with
# router weights pre-multiplied into rh. Router runs token-major ([128 tok,
# 4 experts] PSUM via h-chunk-stationary matmuls); top-2 mask from pairwise
# min/max on DVE; masked weights transposed back via PE transpose and
# broadcast to [128, 512] scale tiles by selector matmuls. Biases enter as
# per-partition ACT bias columns (sb1/rb1/b_in/sb2) and an exp(routb)
# correction factor; all-zero rb2 skips its matmul.

import numpy as np

N_TOTAL = 262144
D = 64
L = 3
NCORES = 8
NPC = N_TOTAL // NCORES  # tokens per core (32768)
MB = 8192                # tokens per macro-batch (h ping/pong resident)
NMB = NPC // MB          # 4
TT = 512                 # tokens per tile
TPM = MB // TT           # 16 tiles per macro-batch
GRP = 4                  # tiles per router group
NG = TPM // GRP          # router groups per macro-batch

_CACHE = {}


def _build_module(with_rb2: bool, npc: int = NPC, mbsz: int = MB, ncores: int = NCORES):
    NPC = npc
    MB = mbsz
    NMB = NPC // MB
    TPM = MB // TT
    NG = TPM // GRP
    from contextlib import ExitStack

    import concourse.bass as bass
    import concourse.tile as tile
    from concourse import bacc, mybir

    f32 = mybir.dt.float32
    f16 = mybir.dt.float16
    AF = mybir.ActivationFunctionType
    OP = mybir.AluOpType

    nc = bacc.Bacc("TRN2", num_devices=ncores, debug=False, enable_asserts=False)

    xin = nc.dram_tensor("xin", [2, NPC], f16, kind="ExternalInput").ap()
    w1c = nc.dram_tensor("w1c", [L, 64, 384], f16, kind="ExternalInput").ap()
    w2c = nc.dram_tensor("w2c", [L, 128, 192], f16, kind="ExternalInput").ap()
    rtc = nc.dram_tensor("rtc", [L, 64, 4], f16, kind="ExternalInput").ap()
    rb2c = nc.dram_tensor("rb2c", [L, 2, 128, 64], f16, kind="ExternalInput").ap()
    # bias columns: 0: b_in; 1+3l: sb1[l]; 2+3l: rb1[l,0:2]; 3+3l: rb1[l,2:4];
    # 10+l: sb2[l].sum(0); 13: b_out (replicated)
    bcl = nc.dram_tensor("bcl", [128, 14], f32, kind="ExternalInput").ap()
    # exp(routb[l]) replicated over partitions
    crw = nc.dram_tensor("crw", [128, 4 * L], f32, kind="ExternalInput").ap()
    winc = nc.dram_tensor("winc", [2, 64], f16, kind="ExternalInput").ap()
    woutc = nc.dram_tensor("woutc", [64, 32], f16, kind="ExternalInput").ap()
    idc = nc.dram_tensor("idc", [128, 128], f32, kind="ExternalInput").ap()
    y = nc.dram_tensor("y", [NPC, 1], f32, kind="ExternalOutput").ap()
    yv = y.rearrange("(a b) o -> a (b o)", b=min(4 * TT, NPC))  # head output rows

    with tile.TileContext(nc) as tc, ExitStack() as ctx:
        singles = ctx.enter_context(tc.tile_pool(name="singles", bufs=1))
        hpool = ctx.enter_context(tc.tile_pool(name="hpool", bufs=2))
        xtpool = ctx.enter_context(tc.tile_pool(name="xtpool", bufs=2))
        stage = ctx.enter_context(tc.tile_pool(name="stage", bufs=3))
        rstage = ctx.enter_context(tc.tile_pool(name="rstage", bufs=2))
        small = ctx.enter_context(tc.tile_pool(name="small", bufs=4))
        ps = ctx.enter_context(tc.tile_pool(name="ps", bufs=1, space="PSUM"))
        dpool = ctx.enter_context(tc.tile_pool(name="dpool", bufs=4, space="DRAM"))

        # --- constants to SBUF (once) ---
        w1_sb = []
        w2_sb = []
        rt_sb = []
        rb2_sb = []
        for l in range(L):
            wl = singles.tile([64, 384], f16, name=f"w1l{l}")
            nc.sync.dma_start(out=wl, in_=w1c[l])
            w1_sb.append(wl)
            w2l = singles.tile([128, 192], f16, name=f"w2l{l}")
            nc.sync.dma_start(out=w2l, in_=w2c[l])
            w2_sb.append(w2l)
            rtl = singles.tile([64, 4], f16, name=f"rtl{l}")
            nc.sync.dma_start(out=rtl, in_=rtc[l])
            rt_sb.append(rtl)
            if with_rb2:
                rbl = singles.tile([128, 2, 64], f16, name=f"rbl{l}")
                nc.sync.dma_start(
                    out=rbl, in_=rb2c[l].rearrange("a p f -> p a f")
                )
                rb2_sb.append(rbl)
        bcl_sb = singles.tile([128, 14], f32)
        nc.sync.dma_start(out=bcl_sb, in_=bcl)
        crw_sb = singles.tile([128, 4 * L], f32)
        nc.sync.dma_start(out=crw_sb, in_=crw)
        win_sb = singles.tile([2, 64], f16)
        nc.sync.dma_start(out=win_sb, in_=winc)
        wout_sb = singles.tile([64, 32], f16)
        nc.sync.dma_start(out=wout_sb, in_=woutc)
        id_sb = singles.tile([128, 128], f32)
        nc.sync.dma_start(out=id_sb, in_=idc)

        for mb in range(NMB):
            mbs = slice(mb * MB, (mb + 1) * MB)
            xt = xtpool.tile([2, MB], f16, tag="xt")
            nc.sync.dma_start(out=xt, in_=xin[:, mbs])

            # ---- layer 0: h = tanh(W_in^T @ [x;t] + b_in) ----
            h = hpool.tile([64, MB], f16, tag="h")
            for t in range(TPM):
                tsl = bass.ts(t, TT)
                p0 = ps.tile([64, TT], f32, tag="po", bufs=1, padded_shape=[128, TT])
                nc.tensor.matmul(p0, win_sb, xt[:, tsl], start=True, stop=True)
                nc.scalar.activation(
                    h[:, tsl], p0, AF.Tanh, bias=bcl_sb[0:64, 0:1]
                )

            # ---- MoE layers ----
            for l in range(L):
                hn = hpool.tile([64, MB], f16, tag="h")
                for g in range(NG):
                    # ===== router for tiles [4g, 4g+4): token-major =====
                    plg = ps.tile([128, GRP * 16], f32, tag="lgwt", bufs=2,
                                  padded_shape=[128, TT])
                    for tg in range(GRP):
                        t = g * GRP + tg
                        for c in range(4):
                            hc = h[:, t * TT + c * 128 : t * TT + (c + 1) * 128]
                            nc.tensor.matmul(
                                plg[:, tg * 16 + c * 4 : tg * 16 + (c + 1) * 4],
                                hc,
                                rt_sb[l][:, :],
                                start=True,
                                stop=True,
                            )
                    ee = rstage.tile([128, GRP * 16], f32, tag="ee")
                    nc.scalar.activation(ee, plg, AF.Exp)
                    # multiply by exp(routb) and sum over experts
                    e3 = ee.rearrange("p (q e) -> p q e", e=4)
                    crw_b = (
                        crw_sb[:, 4 * l : 4 * l + 4]
                        .unsqueeze(1)
                        .broadcast_to((128, GRP * 4, 4))
                    )
                    ec = rstage.tile([128, GRP * 16], f32, tag="ec")
                    ec3 = ec.rearrange("p (q e) -> p q e", e=4)
                    nc.vector.tensor_mul(ec3, e3, crw_b)
                    ss = small.tile([128, GRP * 4], f32, tag="ss")
                    nc.vector.reduce_sum(ss, ec3, axis=mybir.AxisListType.X)
                    rs = small.tile([128, GRP * 4], f32, tag="rs")
                    nc.vector.reciprocal(rs, ss)
                    rw = rstage.tile([128, GRP * 16], f32, tag="rw")
                    rs_b = rs.unsqueeze(2).broadcast_to((128, GRP * 4, 4))
                    r3 = rw.rearrange("p (q e) -> p q e", e=4)
                    nc.vector.tensor_mul(r3, ec3, rs_b)
                    m1 = small.tile([128, GRP * 4], f32, tag="m1")
                    nc.vector.tensor_tensor(m1, r3[:, :, 0], r3[:, :, 1], op=OP.max)
                    n1 = small.tile([128, GRP * 4], f32, tag="n1")
                    nc.vector.tensor_tensor(n1, r3[:, :, 0], r3[:, :, 1], op=OP.min)
                    m2 = small.tile([128, GRP * 4], f32, tag="m2")
                    nc.vector.tensor_tensor(m2, r3[:, :, 2], r3[:, :, 3], op=OP.max)
                    n2 = small.tile([128, GRP * 4], f32, tag="n2")
                    nc.vector.tensor_tensor(n2, r3[:, :, 2], r3[:, :, 3], op=OP.min)
                    t1 = small.tile([128, GRP * 4], f32, tag="t1")
                    nc.vector.tensor_tensor(t1, m1, m2, op=OP.min)
                    t2 = small.tile([128, GRP * 4], f32, tag="t2")
                    nc.vector.tensor_tensor(t2, n1, n2, op=OP.max)
                    snd = small.tile([128, GRP * 4], f32, tag="snd")
                    nc.vector.tensor_tensor(snd, t1, t2, op=OP.max)
                    mk = rstage.tile([128, GRP * 16], f32, tag="mk")
                    snd_b = snd.unsqueeze(2).broadcast_to((128, GRP * 4, 4))
                    nc.vector.tensor_tensor(
                        mk.rearrange("p (q e) -> p q e", e=4), r3, snd_b, op=OP.is_ge
                    )
                    wf = rstage.tile([128, GRP * 16], f32, tag="wf")
                    nc.vector.tensor_mul(wf, rw, mk)

                    # ===== main per-tile compute =====
                    for tg in range(GRP):
                        t = g * GRP + tg
                        tsl = bass.ts(t, TT)
                        hs = h[:, tsl]
                        # W1 stage: 2 shared + 4 routed first-layer matmuls
                        psh = ps.tile([128, TT], f32, tag="w1", bufs=5)
                        nc.tensor.matmul(
                            psh, w1_sb[l][:, 0:128], hs, start=True, stop=True
                        )
                        pr1 = ps.tile([128, TT], f32, tag="w1", bufs=5)
                        nc.tensor.matmul(
                            pr1, w1_sb[l][:, 128:256], hs, start=True, stop=True
                        )
                        pr2 = ps.tile([128, TT], f32, tag="w1", bufs=5)
                        nc.tensor.matmul(
                            pr2, w1_sb[l][:, 256:384], hs, start=True, stop=True
                        )
                        sh = stage.tile([128, 3 * TT], f16, tag="sh")
                        nc.scalar.activation(
                            sh[:, 0:TT], psh, AF.Tanh,
                            bias=bcl_sb[:, 1 + 3 * l : 2 + 3 * l],
                        )
                        nc.scalar.activation(
                            sh[:, TT : 2 * TT], pr1, AF.Tanh,
                            bias=bcl_sb[:, 2 + 3 * l : 3 + 3 * l],
                        )
                        nc.scalar.activation(
                            sh[:, 2 * TT : 3 * TT], pr2, AF.Tanh,
                            bias=bcl_sb[:, 3 + 3 * l : 4 + 3 * l],
                        )

                        # transpose masked weights [128,16] -> [16,128]:
                        # row (4c+e) = chunk c's 128 tokens for expert e
                        pwt = ps.tile([16, 128], f32, tag="lgwt", bufs=2,
                                      padded_shape=[128, TT])
                        nc.tensor.transpose(
                            pwt, wf[:, tg * 16 : (tg + 1) * 16], id_sb
                        )
                        wts = rstage.tile([16, 128], f16, tag="wts")
                        nc.vector.tensor_copy(wts, pwt)
                        # bounce w rows through DRAM, then broadcast-read with
                        # step-0 partition + chunk-strided APs (1 DMA / expert)
                        wdr = dpool.tile([16, 128], f16, tag="wdr")
                        nc.sync.dma_start(out=wdr, in_=wts)
                        pwsb = stage.tile([128, 2 * TT], f16, tag="pwsb")
                        def _brd(e):
                            ap = wdr[0:1, 0:1]
                            return bass.AP(
                                tensor=ap.tensor,
                                offset=ap.offset + e * 128,
                                ap=[[0, 64], [512, 4], [1, 128]],
                            )
                        nc.sync.dma_start(out=pwsb[0:64, 0:TT], in_=_brd(0))
                        nc.sync.dma_start(out=pwsb[64:128, 0:TT], in_=_brd(1))
                        nc.sync.dma_start(out=pwsb[0:64, TT : 2 * TT], in_=_brd(2))
                        nc.sync.dma_start(out=pwsb[64:128, TT : 2 * TT], in_=_brd(3))
                        rsc = stage.tile([128, 2 * TT], f16, tag="rsc")
                        nc.vector.tensor_mul(rsc, sh[:, TT : 3 * TT], pwsb)

                        # W2 stage: accumulate shared + routed (+ rb2)
                        po = ps.tile([64, TT], f32, tag="po", bufs=1,
                                     padded_shape=[128, TT])
                        nc.tensor.matmul(
                            po, w2_sb[l][:, 0:64], sh[:, 0:TT],
                            start=True, stop=False,
                        )
                        nc.tensor.matmul(
                            po, w2_sb[l][:, 64:128], rsc[:, 0:TT],
                            start=False, stop=False,
                        )
                        nc.tensor.matmul(
                            po, w2_sb[l][:, 128:192], rsc[:, TT : 2 * TT],
                            start=False, stop=not with_rb2,
                        )
                        if with_rb2:
                            nc.tensor.matmul(
                                po, rb2_sb[l][:, 0, :], pwsb[:, 0:TT],
                                start=False, stop=False,
                            )
                            nc.tensor.matmul(
                                po, rb2_sb[l][:, 1, :], pwsb[:, TT : 2 * TT],
                                start=False, stop=True,
                            )
                        # residual add on DVE, then tanh
                        ha = stage.tile([64, TT], f32, tag="ha")
                        nc.vector.tensor_add(ha, po, h[:, tsl])
                        nc.scalar.activation(
                            hn[:, tsl], ha, AF.Tanh,
                            bias=bcl_sb[0:64, 10 + l : 11 + l],
                        )
                h = hn

            # ---- head: pack 4 tiles' [1,TT] outputs at partitions {0,32,64,96}
            for hg in range(TPM // 4):
                py = ps.tile([128, TT], f32, tag="po", bufs=1)
                for j in range(4):
                    t = hg * 4 + j
                    tsl = bass.ts(t, TT)
                    nc.tensor.matmul(
                        py[32 * j : 32 * j + 32, :], wout_sb, h[:, tsl],
                        start=True, stop=True, tile_position=(0, 32 * j),
                    )
                ysb = rstage.tile([128, TT], f32, tag="ysb")
                nc.vector.tensor_scalar_add(ysb, py, bcl_sb[:, 13:14])
                yrow = ysb.rearrange("(a b) f -> a b f", b=32)[:, 0, :]  # [4, TT]
                nc.sync.dma_start(
                    out=yv[mb * (TPM // 4) + hg : mb * (TPM // 4) + hg + 1, :],
                    in_=yrow,
                )

    nc.compile()
    return nc


HMB = MB // 2      # tokens per macro-batch half (4096)
TPH = HMB // TT    # ptiles per macro-batch (8); each ptile = 1024 tokens
PGRP = 4           # ptiles per router group
NPG = TPH // PGRP  # router groups per macro-batch (2)


def _build_fast(npc: int = NPC, ncores: int = NCORES, use_fp16: bool = True):
    """Fast path for the all-zero-bias case (the shipped reference).

    Dual-half layout: each [128, x] tile holds tokens of macro-batch half A
    on partitions 0-63 and half B on partitions 64-127 (features 0-63 each).
    W1 / router / input / head matmuls run as row-group-concurrent pairs
    (tile_position rows 0 and 64) with weights duplicated on both partition
    halves. The three W1 outputs per half land in one [128, 3072] PSUM
    6-bank tile -> a single fused tanh. Router weight broadcast goes
    PE-transpose-free: wf -> DMA-transpose -> DRAM bounce ->
    partition-step-0 broadcast reads.
    """
    NPC = npc
    NMB = NPC // MB
    from contextlib import ExitStack

    import concourse.bass as bass
    import concourse.tile as tile
    from concourse import bacc, mybir

    f32 = mybir.dt.float32
    bf16 = mybir.dt.float16 if use_fp16 else mybir.dt.bfloat16
    AF = mybir.ActivationFunctionType
    OP = mybir.AluOpType

    nc = bacc.Bacc("TRN2", num_devices=ncores, debug=False, enable_asserts=False)

    xin = nc.dram_tensor("xin", [4, NPC // 2], bf16, kind="ExternalInput").ap()
    w1c = nc.dram_tensor("w1c", [L, 128, 384], bf16, kind="ExternalInput").ap()
    w2c = nc.dram_tensor("w2c", [L, 128, 192], bf16, kind="ExternalInput").ap()
    # block-diagonal A|B merges: router [0:64,0:4]=routW,[64:128,4:8]=routW;
    # input [0:2,0:64]=W_in,[2:4,64:128]=W_in; head [0:64,0:32],[64:128,32:64]
    rtc = nc.dram_tensor("rtc", [L, 128, 8], bf16, kind="ExternalInput").ap()
    winc = nc.dram_tensor("winc", [4, 128], bf16, kind="ExternalInput").ap()
    woutc = nc.dram_tensor("woutc", [128, 64], bf16, kind="ExternalInput").ap()
    y = nc.dram_tensor("y", [NPC, 1], f32, kind="ExternalOutput").ap()

    with tile.TileContext(nc) as tc, ExitStack() as ctx:
        singles = ctx.enter_context(tc.tile_pool(name="singles", bufs=1))
        hpool = ctx.enter_context(tc.tile_pool(name="hpool", bufs=2))
        xtpool = ctx.enter_context(tc.tile_pool(name="xtpool", bufs=2))
        stage = ctx.enter_context(tc.tile_pool(name="stage", bufs=3))
        rstage = ctx.enter_context(tc.tile_pool(name="rstage", bufs=2))
        small = ctx.enter_context(tc.tile_pool(name="small", bufs=4))
        ps = ctx.enter_context(tc.tile_pool(name="ps", bufs=1, space="PSUM"))
        dpool = ctx.enter_context(tc.tile_pool(name="dpool", bufs=4, space="DRAM"))

        w1_sb = []
        w2_sb = []
        rt_sb = []
        for l in range(L):
            wl = singles.tile([128, 384], bf16, name=f"w1l{l}")
            nc.sync.dma_start(out=wl, in_=w1c[l])
            w1_sb.append(wl)
            w2l = singles.tile([128, 192], bf16, name=f"w2l{l}")
            nc.sync.dma_start(out=w2l, in_=w2c[l])
            w2_sb.append(w2l)
            rtl = singles.tile([128, 8], bf16, name=f"rtl{l}")
            nc.sync.dma_start(out=rtl, in_=rtc[l])
            rt_sb.append(rtl)
        win_sb = singles.tile([4, 128], bf16)
        nc.sync.dma_start(out=win_sb, in_=winc)
        wout_sb = singles.tile([128, 64], bf16)
        nc.sync.dma_start(out=wout_sb, in_=woutc)

        for mb in range(NMB):
            mbs = slice(mb * HMB, (mb + 1) * HMB)
            xt = xtpool.tile([4, HMB], bf16, tag="xt")
            nc.sync.dma_start(out=xt, in_=xin[:, mbs])

            # ---- layer 0: h = tanh(W_in^T @ [x;t]), A|B in one K=4 matmul
            hb = hpool.tile([128, HMB], bf16, tag="hb")
            for t in range(TPH):
                tsl = bass.ts(t, TT)
                p0 = ps.tile([128, TT], f32, tag="po", bufs=1)
                nc.tensor.matmul(p0, win_sb, xt[:, tsl], start=True, stop=True)
                nc.scalar.activation(hb[:, tsl], p0, AF.Tanh)

            # ---- MoE layers ----
            for l in range(L):
                hbn = hpool.tile([128, HMB], bf16, tag="hb")
                for g in range(NPG):
                    # ===== router for ptile group: token-major logits.
                    # One K=128 matmul per chunk computes BOTH halves via the
                    # block-diagonal rt8 (cols 0-3 = A experts, 4-7 = B).
                    plg = ps.tile([128, PGRP * 32], f32, tag="lgwt", bufs=1,
                                  padded_shape=[128, TT])
                    for tg in range(PGRP):
                        t = g * PGRP + tg
                        for c in range(4):
                            csl = slice(t * TT + c * 128, t * TT + (c + 1) * 128)
                            nc.tensor.matmul(
                                plg[:, (tg * 4 + c) * 8 : (tg * 4 + c + 1) * 8],
                                hb[:, csl], rt_sb[l],
                                start=True, stop=True,
                            )
                    ee = rstage.tile([128, PGRP * 32], f32, tag="ee")
                    nc.scalar.activation(ee, plg, AF.Exp)
                    e4 = ee.rearrange("p (q e) -> p q e", e=4)
                    ss = small.tile([128, PGRP * 8], f32, tag="ss")
                    nc.vector.reduce_sum(ss, e4, axis=mybir.AxisListType.X)
                    rs = small.tile([128, PGRP * 8], f32, tag="rs")
                    nc.vector.reciprocal(rs, ss)
                    rw = rstage.tile([128, PGRP * 32], f32, tag="rw")
                    rs_b = rs.unsqueeze(2).broadcast_to((128, PGRP * 8, 4))
                    r4 = rw.rearrange("p (q e) -> p q e", e=4)
                    nc.vector.tensor_mul(r4, e4, rs_b)
                    m1 = small.tile([128, PGRP * 8], f32, tag="m1")
                    nc.vector.tensor_tensor(m1, r4[:, :, 0], r4[:, :, 1], op=OP.max)
                    n1 = small.tile([128, PGRP * 8], f32, tag="n1")
                    nc.vector.tensor_tensor(n1, r4[:, :, 0], r4[:, :, 1], op=OP.min)
                    m2 = small.tile([128, PGRP * 8], f32, tag="m2")
                    nc.vector.tensor_tensor(m2, r4[:, :, 2], r4[:, :, 3], op=OP.max)
                    n2 = small.tile([128, PGRP * 8], f32, tag="n2")
                    nc.vector.tensor_tensor(n2, r4[:, :, 2], r4[:, :, 3], op=OP.min)
                    t1 = small.tile([128, PGRP * 8], f32, tag="t1")
                    nc.vector.tensor_tensor(t1, m1, m2, op=OP.min)
                    t2 = small.tile([128, PGRP * 8], f32, tag="t2")
                    nc.vector.tensor_tensor(t2, n1, n2, op=OP.max)
                    snd = small.tile([128, PGRP * 8], f32, tag="snd")
                    nc.vector.tensor_tensor(snd, t1, t2, op=OP.max)
                    mk = rstage.tile([128, PGRP * 32], f32, tag="mk")
                    snd_b = snd.unsqueeze(2).broadcast_to((128, PGRP * 8, 4))
                    nc.vector.tensor_tensor(
                        mk.rearrange("p (q e) -> p q e", e=4), r4, snd_b, op=OP.is_ge
                    )
                    wf = rstage.tile([128, PGRP * 32], bf16, tag="wf")
                    nc.vector.tensor_mul(wf, rw, mk)
                    # token-major -> row-per-(tile,half,chunk,expert) via DMA
                    # transpose, then DRAM bounce for broadcast reads
                    wtg = rstage.tile([128, 128], bf16, tag="wtg")
                    nc.sync.dma_start_transpose(wtg, wf)
                    wdr = dpool.tile([128, 128], bf16, tag="wdr")
                    nc.sync.dma_start(out=wdr, in_=wtg)

                    # ===== per-ptile compute =====
                    for tg in range(PGRP):
                        t = g * PGRP + tg
                        tsl = bass.ts(t, TT)
                        # broadcast masked weights to [128, 4*TT] scale tiles
                        # wdr row r = (tg*4+c)*8 + half*4 + e
                        pwsb = stage.tile([128, 4 * TT], bf16, tag="pwsb")
                        def _brd(half, e):
                            ap = wdr[0:1, 0:1]
                            return bass.AP(
                                tensor=ap.tensor,
                                offset=ap.offset + (tg * 32 + half * 4 + e) * 128,
                                ap=[[0, 64], [1024, 4], [1, 128]],
                            )
                        for half in range(2):
                            for pr in range(2):
                                blk = half * 2 + pr
                                nc.sync.dma_start(
                                    out=pwsb[0:64, blk * TT : (blk + 1) * TT],
                                    in_=_brd(half, 2 * pr),
                                )
                                nc.sync.dma_start(
                                    out=pwsb[64:128, blk * TT : (blk + 1) * TT],
                                    in_=_brd(half, 2 * pr + 1),
                                )

                        # W1 stage: 3 row-group-concurrent pairs -> 6 banks
                        psw = ps.tile([128, 3072], f32, tag="w1", bufs=1)
                        for j in range(3):
                            jsl = slice(j * 128, (j + 1) * 128)
                            nc.tensor.matmul(
                                psw[:, j * TT : (j + 1) * TT],
                                w1_sb[l][0:64, jsl], hb[0:64, tsl],
                                start=True, stop=True, tile_position=(0, 0),
                            )
                            nc.tensor.matmul(
                                psw[:, 1536 + j * TT : 1536 + (j + 1) * TT],
                                w1_sb[l][64:128, jsl], hb[64:128, tsl],
                                start=True, stop=True, tile_position=(64, 0),
                            )
                        sh = stage.tile([128, 3072], bf16, tag="sh")
                        nc.scalar.activation(sh, psw, AF.Tanh)

                        rsc = stage.tile([128, 2048], bf16, tag="rsc")
                        nc.vector.tensor_mul(
                            rsc[:, 0:1024], sh[:, TT : 3 * TT], pwsb[:, 0 : 2 * TT]
                        )
                        nc.vector.tensor_mul(
                            rsc[:, 1024:2048], sh[:, 2048:3072],
                            pwsb[:, 2 * TT : 4 * TT],
                        )

                        # W2 stage: A -> po[0:64], B -> po[64:128]
                        po = ps.tile([128, TT], f32, tag="po", bufs=1)
                        nc.tensor.matmul(
                            po[0:64, :], w2_sb[l][:, 0:64], sh[:, 0:TT],
                            start=True, stop=False, tile_position=(0, 0),
                        )
                        nc.tensor.matmul(
                            po[0:64, :], w2_sb[l][:, 64:128], rsc[:, 0:TT],
                            start=False, stop=False, tile_position=(0, 0),
                        )
                        nc.tensor.matmul(
                            po[0:64, :], w2_sb[l][:, 128:192], rsc[:, TT : 2 * TT],
                            start=False, stop=True, tile_position=(0, 0),
                        )
                        nc.tensor.matmul(
                            po[64:128, :], w2_sb[l][:, 0:64], sh[:, 1536:2048],
                            start=True, stop=False, tile_position=(0, 64),
                        )
                        nc.tensor.matmul(
                            po[64:128, :], w2_sb[l][:, 64:128], rsc[:, 2 * TT : 3 * TT],
                            start=False, stop=False, tile_position=(0, 64),
                        )
                        nc.tensor.matmul(
                            po[64:128, :], w2_sb[l][:, 128:192], rsc[:, 3 * TT : 4 * TT],
                            start=False, stop=True, tile_position=(0, 64),
                        )
                        # residual add on DVE, tanh back to 16-bit h
                        ha = stage.tile([128, TT], f32, tag="ha")
                        nc.vector.tensor_add(ha, po, hb[:, tsl])
                        nc.scalar.activation(hbn[:, tsl], ha, AF.Tanh)
                hb = hbn

            # ---- head: 2 ptiles per [128, TT] PSUM tile; one K=128 matmul
            # per ptile yields [A y | B y] on partitions 0-31 / 32-63 ----
            for hg in range(TPH // 2):
                py = ps.tile([128, TT], f32, tag="po", bufs=1)
                for j in range(2):
                    t = hg * 2 + j
                    tsl = bass.ts(t, TT)
                    nc.tensor.matmul(
                        py[64 * j : 64 * (j + 1), :], wout_sb, hb[:, tsl],
                        start=True, stop=True, tile_position=(0, 64 * j),
                    )
                ysb = rstage.tile([128, TT], f32, tag="ysb")
                nc.vector.tensor_copy(ysb, py)
                yrow = ysb.rearrange("(a b) f -> a b f", b=32)[:, 0, :]  # [4, TT]
                yt = y[0:1, 0:1]
                ydst = bass.AP(
                    tensor=yt.tensor,
                    offset=yt.offset + mb * MB + hg * 2 * TT,
                    ap=[[TT, 2], [HMB, 2], [1, TT]],
                )
                nc.sync.dma_start(out=ydst, in_=yrow)

    nc.compile()
    return nc


def _prep_host(inputs):
    f = np.float32
    bf = np.float16
    x = np.asarray(inputs["x"], f).reshape(-1)
    t = np.asarray(inputs["t"], f).reshape(-1)
    W_in = np.asarray(inputs["W_in"], f)
    b_in = np.asarray(inputs["b_in"], f)
    sW1 = np.asarray(inputs["sW1"], f)
    sb1 = np.asarray(inputs["sb1"], f)
    sW2 = np.asarray(inputs["sW2"], f)
    sb2 = np.asarray(inputs["sb2"], f)
    rW1 = np.asarray(inputs["rW1"], f)
    rb1 = np.asarray(inputs["rb1"], f)
    rW2 = np.asarray(inputs["rW2"], f)
    rb2 = np.asarray(inputs["rb2"], f)
    routW = np.asarray(inputs["routW"], f)
    routb = np.asarray(inputs["routb"], f)
    W_out = np.asarray(inputs["W_out"], f)
    b_out = np.asarray(inputs["b_out"], f)

    w1c = np.zeros((L, 64, 384), f)
    w2c = np.zeros((L, 128, 192), f)
    rtc = np.zeros((L, 64, 4), f)
    for l in range(L):
        w1c[l, :, 0:128] = np.transpose(sW1[l], (1, 0, 2)).reshape(64, 128)
        w1c[l, :, 128:256] = np.transpose(rW1[l, 0:2], (1, 0, 2)).reshape(64, 128)
        w1c[l, :, 256:384] = np.transpose(rW1[l, 2:4], (1, 0, 2)).reshape(64, 128)
        w2c[l, :, 0:64] = sW2[l].reshape(128, 64)
        w2c[l, :, 64:128] = rW2[l, 0:2].reshape(128, 64)
        w2c[l, :, 128:192] = rW2[l, 2:4].reshape(128, 64)
        rtc[l] = routW[l]
    rb2c = np.zeros((L, 2, 128, 64), f)
    for l in range(L):
        for half in range(2):
            for eb in range(2):
                e = half * 2 + eb
                rb2c[l, half, eb * 64 : (eb + 1) * 64, :] = rb2[l, e][None, :] / 64.0
    bcl = np.zeros((128, 14), f)
    bcl[0:64, 0] = b_in
    for l in range(L):
        bcl[:, 1 + 3 * l] = sb1[l].reshape(128)
        bcl[:, 2 + 3 * l] = rb1[l, 0:2].reshape(128)
        bcl[:, 3 + 3 * l] = rb1[l, 2:4].reshape(128)
        bcl[0:64, 10 + l] = sb2[l].sum(0)
    bcl[:, 13] = b_out[0]
    crw = np.zeros((128, 4 * L), f)
    for l in range(L):
        crw[:, 4 * l : 4 * l + 4] = np.exp(routb[l])[None, :]
    winc = np.ascontiguousarray(W_in)  # [2, 64]
    woutc = np.ascontiguousarray(np.repeat(W_out, 32, axis=1))  # [64, 32]
    idc = np.eye(128, dtype=f)

    shared = {
        "w1c": w1c.astype(bf), "w2c": w2c.astype(bf), "rtc": rtc.astype(bf),
        "rb2c": rb2c.astype(bf), "bcl": bcl, "crw": crw,
        "winc": winc.astype(bf), "woutc": woutc.astype(bf), "idc": idc,
    }
    in_maps = []
    for c in range(NCORES):
        sl = slice(c * NPC, (c + 1) * NPC)
        xin = np.stack([x[sl], t[sl]], 0)  # [2, NPC]
        in_maps.append({"xin": np.ascontiguousarray(xin).astype(bf), **shared})
    return in_maps, bool(np.any(rb2 != 0.0))


def _prep_fast(inputs, use_fp16=True):
    f = np.float32
    bf = np.float16 if use_fp16 else None
    if bf is None:
        import ml_dtypes

        bf = ml_dtypes.bfloat16
    x = np.asarray(inputs["x"], f).reshape(-1)
    t = np.asarray(inputs["t"], f).reshape(-1)
    W_in = np.asarray(inputs["W_in"], f)
    sW1 = np.asarray(inputs["sW1"], f)
    sW2 = np.asarray(inputs["sW2"], f)
    rW1 = np.asarray(inputs["rW1"], f)
    rW2 = np.asarray(inputs["rW2"], f)
    routW = np.asarray(inputs["routW"], f)
    W_out = np.asarray(inputs["W_out"], f)

    w1c = np.zeros((L, 128, 384), f)
    w2c = np.zeros((L, 128, 192), f)
    rtc = np.zeros((L, 128, 8), f)
    for l in range(L):
        w1c[l, 0:64, 0:128] = np.transpose(sW1[l], (1, 0, 2)).reshape(64, 128)
        w1c[l, 0:64, 128:256] = np.transpose(rW1[l, 0:2], (1, 0, 2)).reshape(64, 128)
        w1c[l, 0:64, 256:384] = np.transpose(rW1[l, 2:4], (1, 0, 2)).reshape(64, 128)
        w1c[l, 64:128] = w1c[l, 0:64]
        w2c[l, :, 0:64] = sW2[l].reshape(128, 64)
        w2c[l, :, 64:128] = rW2[l, 0:2].reshape(128, 64)
        w2c[l, :, 128:192] = rW2[l, 2:4].reshape(128, 64)
        rtc[l, 0:64, 0:4] = routW[l]
        rtc[l, 64:128, 4:8] = routW[l]
    winc = np.zeros((4, 128), f)
    winc[0:2, 0:64] = W_in
    winc[2:4, 64:128] = W_in
    woutc = np.zeros((128, 64), f)
    woutc[0:64, 0:32] = np.repeat(W_out, 32, axis=1)
    woutc[64:128, 32:64] = woutc[0:64, 0:32]

    shared = {
        "w1c": w1c.astype(bf), "w2c": w2c.astype(bf), "rtc": rtc.astype(bf),
        "winc": winc.astype(bf), "woutc": woutc.astype(bf),
    }
    in_maps = []
    for c in range(NCORES):
        sl = slice(c * NPC, (c + 1) * NPC)
        # split each macro-batch into halves A/B: rows 0-1 = (x,t) of A
        # tokens, rows 2-3 = B tokens
        xv = x[sl].reshape(NPC // MB, 2, HMB)
        tv = t[sl].reshape(NPC // MB, 2, HMB)
        xin = np.stack([
            xv[:, 0, :].reshape(-1), tv[:, 0, :].reshape(-1),
            xv[:, 1, :].reshape(-1), tv[:, 1, :].reshape(-1),
        ], 0)  # [4, NPC//2]
        in_maps.append({"xin": np.ascontiguousarray(xin).astype(bf), **shared})
    return in_maps


def _fast_ok(inputs):
    for k in ("b_in", "sb1", "sb2", "rb1", "rb2", "routb", "b_out"):
        if np.any(np.asarray(inputs[k], np.float32) != 0.0):
            return False
    return True


def _get_module(kind, *args):
    key = (kind,) + args
    if key not in _CACHE:
        if kind == "fast":
            _CACHE[key] = _build_fast(use_fp16=args[0])
        else:
            _CACHE[key] = _build_module(*args)
    return _CACHE[key]


def _run(inputs, trace=False, use_fp16=True):
    from concourse.bass_utils import run_bass_kernel_spmd

    if _fast_ok(inputs):
        in_maps = _prep_fast(inputs, use_fp16=use_fp16)
        nc = _get_module("fast", use_fp16)
    else:
        in_maps, with_rb2 = _prep_host(inputs)
        nc = _get_module("gen", with_rb2)
    res = run_bass_kernel_spmd(
        nc, in_maps, core_ids=list(range(NCORES)), trace=trace
    )
    yy = np.concatenate([r["y"] for r in res.results], 0).astype(np.float32)
    return yy, res


def kernel(**inputs) -> np.ndarray:
    yy, _ = _run(inputs, trace=False)
    return yy

